# revision 31
# baseline (speedup 1.0000x reference)
"""Fused ARFlow kernel for Trainium2 (8 NeuronCores, data-parallel over batch).

Reference computes three causal K=3 convs (64->256->256->128 ch) with NO
nonlinearity between them, then z = exp(alpha*tanh(ls)+beta)*x + tt.
The convs are linear, so they compose on the host into a single causal K=7
conv (64->128 ch) with an effective bias, exact for t>=4; an x-independent
(weights-only) correction D fixes outputs t<4 where the reference's
zero-padding of *biased* intermediates differs from the composition.

Device kernel per core (4 samples, processed as 2 sample-PAIRS so every
post-matmul op runs at the full 128 partitions):
  - x is loaded per sample as a [128, SW+8] fp16 SUPERTILE (4 compute
    tiles worth) with tap-pair packing (partitions 0-63 = x shifted 8,
    64-127 = x shifted 7; the second sample of a pair uses the SWAPPED
    layout so its data sits in partitions 64-127 wherever the pair-packed
    elementwise ops need it),
  - the K=7 conv is 4 fp16 matmuls of contraction 128 per 512-col chunk;
    outputs are split by weight columns into a "log_s" PSUM tile and a "t"
    PSUM tile, each [128, TS] holding BOTH samples of the pair (64-col
    matmuls auto-col-tile into PE column groups 0/64, which run
    concurrently),
  - ScalarE: TH = tanh(ps_ls + b_ls) [128,TS]; E = exp(TH*alpha+beta) fp16,
  - VectorE: ZM halves = E * x (fp16, 2x mode); then one fused
    scalar_tensor_tensor: zstage = (ps_t + b_t) + ZM,
  - z stored as fp16 (upcast to fp32 on host; well within tolerance) from
    [128, 2*TS] staging chunks as soon as both halves are written.

DMA plan: x supertile loads are ~0.5 MB transfers (8 KB per-partition
lines) -- x2a halves on sync (HWDGE), x2b halves + z stores on gpsimd
(SWDGE); constants are merged into 3 DMAs; supertile-0 zero halos are
memset, and the first pair's x tiles are issued right after the weights so
the first matmul starts early. PE warm-up matmuls run during the head's
DMA wait to flip the HAM clock gate to 8/8 before real work.
"""

import numpy as np

import concourse.bacc as bacc
import concourse.bass as bass
import concourse.mybir as mybir
import concourse.tile as tile
from concourse.bass_utils import run_bass_kernel_spmd

N_CORES = 8
B, C, T = 32, 64, 8192
NS = B // N_CORES          # samples per core
TS = 1024                  # time-tile width (multiple of 512)
SUP = 4                    # compute tiles per DMA supertile
O = 128                    # output channels (2C)

F32 = mybir.dt.float32
F16 = mybir.dt.float16


# ---------------------------------------------------------------- host math

def _compose(w_in, b_in, w_mid, b_mid, w_out, b_out):
    """W_eff (128, 64, 7), b_eff (128,), D (128, 4)."""
    w_in = np.asarray(w_in, np.float64)
    w_mid = np.asarray(w_mid, np.float64)
    w_out = np.asarray(w_out, np.float64)
    b_in = np.asarray(b_in, np.float64)
    b_mid = np.asarray(b_mid, np.float64)
    b_out = np.asarray(b_out, np.float64)
    H = w_in.shape[0]

    w12 = np.zeros((w_mid.shape[0], w_in.shape[1], 5))
    for i in range(3):
        for j in range(3):
            w12[:, :, i + j] += w_mid[:, :, j] @ w_in[:, :, i]
    w_eff = np.zeros((w_out.shape[0], w_in.shape[1], 7))
    for i in range(5):
        for l in range(3):
            w_eff[:, :, i + l] += w_out[:, :, l] @ w12[:, :, i]

    b_eff = w_out.sum(2) @ (w_mid.sum(2) @ b_in + b_mid) + b_out

    # boundary correction: reference chain on x=0, minus steady-state b_eff
    Tz = 12
    rs1 = np.zeros((H, Tz)) + b_in[:, None]
    rs2 = np.zeros((w_mid.shape[0], Tz))
    for t in range(Tz):
        acc = b_mid.copy()
        for j in range(3):
            tau = t - 2 + j
            if tau >= 0:
                acc = acc + w_mid[:, :, j] @ rs1[:, tau]
        rs2[:, t] = acc
    rs3 = np.zeros((w_out.shape[0], Tz))
    for t in range(Tz):
        acc = b_out.copy()
        for l in range(3):
            tau = t - 2 + l
            if tau >= 0:
                acc = acc + w_out[:, :, l] @ rs2[:, tau]
        rs3[:, t] = acc
    D = rs3[:, 0:4] - b_eff[:, None]
    return (w_eff.astype(np.float32), b_eff.astype(np.float32),
            D.astype(np.float32))


def _pack_weights(w_eff):
    """Stationary lhsT tiles as (p, m, o): m 0-2 = tap pairs (1,2),(3,4),
    (5,6) with rows 0-63 = odd tap, 64-127 = even tap; m 4-6 = swapped
    halves; m 8/9 = tap-0 weights for ls/t duplicated into all four 64x64
    quadrants (consumed by the 64x64-tiled quad matmuls)."""
    WT = np.zeros((10, 128, 128), np.float32)
    for m in range(3):
        WT[m, 0:64, :] = w_eff[:, :, 2 * m + 1].T
        WT[m, 64:128, :] = w_eff[:, :, 2 * m + 2].T
        WT[4 + m, 64:128, :] = w_eff[:, :, 2 * m + 1].T
        WT[4 + m, 0:64, :] = w_eff[:, :, 2 * m + 2].T
    q_ls = w_eff[0:64, :, 0].T
    q_t = w_eff[64:128, :, 0].T
    WT[8] = np.block([[q_ls, q_ls], [q_ls, q_ls]])
    WT[9] = np.block([[q_t, q_t], [q_t, q_t]])
    return np.ascontiguousarray(WT.transpose(1, 0, 2))


# ------------------------------------------------------------- device build

def build_nc(ns=NS, t_len=T, ts=TS, sup=SUP):
    assert ts % 512 == 0 and t_len % (ts * sup) == 0
    # supertile plan per pair: small leading supertiles so the first
    # matmul's x DMA is small and arrivals stay ahead of compute while
    # the rings ramp up; full-size ones in steady state
    first_plan = [2, 2, 2, 2]
    rest_plan = [sup] * (t_len // ts // sup)
    assert sum(first_plan) == sum(rest_plan) == t_len // ts
    nc = bacc.Bacc("TRN2", target_bir_lowering=False, debug=False,
                   num_devices=N_CORES)
    xs = nc.dram_tensor("xs", (ns, C, t_len), F16, kind="ExternalInput").ap()
    # host-pretransposed to (p, m, o) so the load is a dense 320 KB DMA
    wt = nc.dram_tensor("wt", (128, 10, 128), F16, kind="ExternalInput").ap()
    # cmat rows: 0-127 = D_ls one-hot lhsT cols, 128-255 = D_t, 256-767 = bind
    cmat = nc.dram_tensor("cmat", (4, 768), F16, kind="ExternalInput").ap()
    # cvec cols: 0 = b_ls, 1 = b_t, 2 = alpha, 3 = beta (dup-packed halves)
    cvec = nc.dram_tensor("cvec", (128, 4), F32, kind="ExternalInput").ap()
    z = nc.dram_tensor("z", (ns, C, t_len), F16, kind="ExternalOutput").ap()

    W4 = sup * ts + 8          # max supertile width incl halo
    pairs = ns // 2
    Tanh = mybir.ActivationFunctionType.Tanh
    Exp = mybir.ActivationFunctionType.Exp
    ADD = mybir.AluOpType.add
    MUL = mybir.AluOpType.mult

    with tile.TileContext(nc) as tc:
        with (
            tc.tile_pool(name="consts", bufs=1) as consts,
            tc.tile_pool(name="data", bufs=2) as data,
            tc.tile_pool(name="outs", bufs=3) as outs,
            tc.tile_pool(name="psum", bufs=2, space="PSUM") as psum_pool,
        ):
            def load_xsuper(x2, s, t0, nsub, swapped, dma_lo, dma_hi):
                # lo rows get x shifted 8, hi rows x shifted 7; a swapped
                # tile exchanges which partition half holds which shift.
                lo = x2[64:128, :] if swapped else x2[0:64, :]
                hi = x2[0:64, :] if swapped else x2[64:128, :]
                w = nsub * ts + 8
                if t0 == 0:
                    nc.gpsimd.memset(lo[:, 0:8], 0.0)
                    nc.gpsimd.memset(hi[:, 0:7], 0.0)
                    dma_lo(lo[:, 8:w], xs[s, :, 0:w - 8])
                    dma_hi(hi[:, 7:w - 1], xs[s, :, 0:w - 8])
                else:
                    dma_lo(lo[:, 0:w], xs[s, :, t0 - 8:t0 + nsub * ts])
                    dma_hi(hi[:, 0:w - 1], xs[s, :, t0 - 7:t0 + nsub * ts])

            def make_x2(pr, t0, nsub):
                # one buffer per supertile: loads and halo memsets never
                # wait on buffer reuse, so they can never block the DMA
                # queues behind them. (The ACT queue must stay free of DMA
                # dispatches - they'd delay the tanh/exp chain.)
                x2a = data.tile([128, W4], F16, tag="x2a", name="x2a",
                                bufs=7)
                x2b = data.tile([128, W4], F16, tag="x2b", name="x2b",
                                bufs=7)
                load_xsuper(x2a, 2 * pr, t0, nsub, False,
                            nc.sync.dma_start, nc.sync.dma_start)
                load_xsuper(x2b, 2 * pr + 1, t0, nsub, True,
                            nc.gpsimd.dma_start, nc.gpsimd.dma_start)
                return x2a, x2b

            # supertile schedule: (pair, t0, nsub)
            supers = []
            for pr in range(pairs):
                plan = first_plan if pr == 0 else rest_plan
                t0 = 0
                for nsub in plan:
                    supers.append((pr, t0, nsub))
                    t0 += nsub * ts

            # weights first (the first matmul's only const dependency),
            # then the first supertiles on all three DMA rings; cmat/cvec
            # (needed only ~2us into the first iteration) go last
            wt_sb = consts.tile([128, 10, 128], F16)
            nc.sync.dma_start(wt_sb, wt)
            pre = [make_x2(*supers[0])]
            pre.append(make_x2(*supers[1]))
            cm_sb = consts.tile([4, 768], F16)
            nc.gpsimd.dma_start(cm_sb, cmat)
            cv_sb = consts.tile([128, 4], F32)
            nc.sync.dma_start(cv_sb, cvec)

            # PE warm-up: ~4.5us of throwaway matmuls during the head's
            # DMA wait flips the HAM clock gate to 8/8 before real work
            warm = data.tile([128, 512], F16, tag="warm", name="warm",
                             bufs=1)
            nc.vector.memset(warm, 0.0)
            warm_ps = psum_pool.tile([128, 512], F32, tag="psls",
                                     name="warm_ps")
            for _ in range(9):
                nc.tensor.matmul(warm_ps[:, 0:512], warm[:, 0:128],
                                 warm[:, 0:512], start=True, stop=True,
                                 skip_group_check=True)
            bls_sb = cv_sb[:, 0:1]
            btt_sb = cv_sb[:, 1:2]
            avec_sb = cv_sb[:, 2:3]
            bevec_sb = cv_sb[:, 3:4]
            dls_sb = cm_sb[:, 0:128]
            dtt_sb = cm_sb[:, 128:256]
            bind_sb = cm_sb[:, 256:768]

            xq = dict(enumerate(pre))
            zst = None
            for k, (pr, st0, nsub) in enumerate(supers):
                # prefetch upcoming supertiles' x while this one computes
                for kf in (k + 1, k + 2, k + 3):
                    if kf < len(supers) and kf not in xq:
                        xq[kf] = make_x2(*supers[kf])
                x2a, x2b = xq.pop(k)
                s0, s1 = 2 * pr, 2 * pr + 1

                for q in range(nsub):
                    it = st0 // ts + q     # tile index within the pair
                    o = q * ts             # column offset in the supertile
                    t0 = it * ts

                    ps_ls = psum_pool.tile([128, ts], F32, tag="psls",
                                           name="ps_ls")
                    ps_t = psum_pool.tile([128, ts], F32, tag="pst",
                                          name="ps_t")
                    corr = (it == 0)

                    # tap-0 runs as four concurrent 64x64-tiled K=64
                    # matmuls (2 row-bands = h-chunks x 2 col-bands =
                    # samples). The ls quad OPENS its psum (start=True,
                    # iteration head) and the t quad CLOSES its psum
                    # (stop=True, iteration tail): ps_ls still completes
                    # as early as possible for the tanh chain, while
                    # ps_t's first write happens as LATE as possible so
                    # the 2-deep psum rotation has maximal slack. Only 2
                    # tile-mode switches per iteration this way.
                    def quad(ps, mq, start, stop_h0, stop_h1):
                        w6 = wt_sb[:, mq, :]
                        nc.tensor.matmul(
                            ps[0:64, 0:512], w6[0:64, 0:64],
                            x2a[0:64, o + 1:o + 513],
                            start=start, stop=stop_h0)
                        nc.tensor.matmul(
                            ps[64:128, 0:512], w6[64:128, 64:128],
                            x2b[64:128, o + 1:o + 513],
                            start=start, stop=stop_h0)
                        nc.tensor.matmul(
                            ps[0:64, 512:1024], w6[64:128, 0:64],
                            x2a[64:128, o + 512:o + 1024],
                            start=start, stop=stop_h1)
                        nc.tensor.matmul(
                            ps[64:128, 512:1024], w6[0:64, 64:128],
                            x2b[0:64, o + 512:o + 1024],
                            start=start, stop=stop_h1)

                    def pairs(ps, o0, start, stop_last):
                        for m in range(3):
                            for h in range(ts // 512):
                                ho = h * 512
                                c0 = o + ho + 2 * m + 2
                                stop = (stop_last and m == 2
                                        and not (corr and h == 0))
                                nc.tensor.matmul(
                                    ps[0:64, ho:ho + 512],
                                    wt_sb[:, m, o0:o0 + 64],
                                    x2a[:, c0:c0 + 512],
                                    start=(start and m == 0), stop=stop)
                                nc.tensor.matmul(
                                    ps[64:128, ho:ho + 512],
                                    wt_sb[:, 4 + m, o0:o0 + 64],
                                    x2b[:, c0:c0 + 512],
                                    start=(start and m == 0), stop=stop)

                    def corr_mm(ps, o0):
                        nc.tensor.matmul(
                            ps[:, 0:512],
                            dls_sb if o0 == 0 else dtt_sb,
                            bind_sb, start=False, stop=True,
                            skip_group_check=True)

                    quad(ps_ls, 8, start=True, stop_h0=False,
                         stop_h1=False)
                    pairs(ps_ls, 0, start=False, stop_last=True)
                    if corr:
                        corr_mm(ps_ls, 0)
                    pairs(ps_t, 64, start=True, stop_last=False)
                    quad(ps_t, 9, start=False, stop_h0=not corr,
                         stop_h1=True)
                    if corr:
                        corr_mm(ps_t, 64)

                    th = outs.tile([128, ts], F16, tag="th", name="th")
                    nc.scalar.activation(th, ps_ls, Tanh, bias=bls_sb)
                    e = outs.tile([128, ts], F16, tag="e", name="e")
                    nc.scalar.activation(e, th, Exp,
                                         bias=bevec_sb, scale=avec_sb)
                    zm = outs.tile([128, ts], F16, tag="zm", name="zm")
                    nc.vector.tensor_tensor(
                        zm[0:64, :], e[0:64, :],
                        x2a[0:64, 8 + o:8 + o + ts], MUL)
                    nc.vector.tensor_tensor(
                        zm[64:128, :], e[64:128, :],
                        x2b[64:128, 8 + o:8 + o + ts], MUL)
                    # fused: zstage = (ps_t + b_t) + zm  (one DVE op)
                    if it % 2 == 0:
                        zst = outs.tile([128, 2 * ts], F16, tag="zst",
                                        name="zst")
                    zo = (it % 2) * ts
                    nc.vector.scalar_tensor_tensor(
                        zst[:, zo:zo + ts], ps_t, btt_sb, zm, ADD, ADD)
                    if it >= t_len // ts - 2:
                        # final two tiles: store each 1024-col chunk the
                        # moment it is ready, halving the tail drain
                        nc.sync.dma_start(z[s0, :, t0:t0 + ts],
                                          zst[0:64, zo:zo + ts])
                        nc.gpsimd.dma_start(z[s1, :, t0:t0 + ts],
                                            zst[64:128, zo:zo + ts])
                    elif it % 2 == 1:
                        # split stores across the two DMA queues; loads for
                        # upcoming supertiles are issued ahead of these in
                        # program order so the zstage-ready wait here never
                        # delays a load dispatch
                        tst = t0 - ts
                        nc.sync.dma_start(z[s0, :, tst:tst + 2 * ts],
                                          zst[0:64, :])
                        nc.gpsimd.dma_start(z[s1, :, tst:tst + 2 * ts],
                                            zst[64:128, :])

    nc.compile()
    return nc


def make_in_maps(x, w_in, b_in, w_mid, b_mid, w_out, b_out, alpha, beta,
                 n_cores=N_CORES):
    w_eff, b_eff, D = _compose(w_in, b_in, w_mid, b_mid, w_out, b_out)
    WT = _pack_weights(w_eff)
    CMAT = np.zeros((4, 768), np.float32)
    CMAT[:, 0:64] = D[0:64, :].T
    CMAT[:, 64:128] = D[0:64, :].T
    CMAT[:, 128:192] = D[64:128, :].T
    CMAT[:, 192:256] = D[64:128, :].T
    for i in range(4):
        CMAT[i, 256 + i] = 1.0
    al = np.asarray(alpha, np.float32).reshape(64)
    be = np.asarray(beta, np.float32).reshape(64)
    CVEC = np.stack([
        np.concatenate([b_eff[0:64], b_eff[0:64]]),
        np.concatenate([b_eff[64:128], b_eff[64:128]]),
        np.concatenate([al, al]),
        np.concatenate([be, be]),
    ], axis=1).astype(np.float32)
    x16 = np.ascontiguousarray(np.asarray(x, np.float32)).astype(np.float16)
    ns = x16.shape[0] // n_cores
    maps = []
    for i in range(n_cores):
        m = dict(xs=np.ascontiguousarray(x16[i * ns:(i + 1) * ns]),
                 wt=WT.astype(np.float16),
                 cmat=CMAT.astype(np.float16),
                 cvec=np.ascontiguousarray(CVEC))
        maps.append(m)
    return maps


_NC_CACHE = {}


def _get_nc():
    if "nc" not in _NC_CACHE:
        _NC_CACHE["nc"] = build_nc()
    return _NC_CACHE["nc"]


def kernel(x, w_in, b_in, w_mid, b_mid, w_out, b_out, alpha, beta,
           _trace=False, _trace_kwargs=None):
    nc = _get_nc()
    in_maps = make_in_maps(x, w_in, b_in, w_mid, b_mid, w_out, b_out,
                           alpha, beta)
    res = run_bass_kernel_spmd(nc, in_maps, core_ids=list(range(N_CORES)),
                               trace=_trace, **(_trace_kwargs or {}))
    out = np.concatenate([r["z"] for r in res.results], axis=0)
    kernel.last_results = res
    return out.astype(np.float32)


# revision 32
# speedup vs baseline: 1.0795x; 1.0795x over previous
"""Fused ARFlow kernel for Trainium2 (8 NeuronCores, data-parallel over batch).

Reference computes three causal K=3 convs (64->256->256->128 ch) with NO
nonlinearity between them, then z = exp(alpha*tanh(ls)+beta)*x + tt.
The convs are linear, so they compose on the host into a single causal K=7
conv (64->128 ch) with an effective bias, exact for t>=4; an x-independent
(weights-only) correction D fixes outputs t<4 where the reference's
zero-padding of *biased* intermediates differs from the composition.

Device kernel per core (4 samples, processed as 2 sample-PAIRS so every
post-matmul op runs at the full 128 partitions):
  - x is loaded per sample as a [128, SW+8] fp16 SUPERTILE (4 compute
    tiles worth) with tap-pair packing (partitions 0-63 = x shifted 8,
    64-127 = x shifted 7; the second sample of a pair uses the SWAPPED
    layout so its data sits in partitions 64-127 wherever the pair-packed
    elementwise ops need it),
  - the K=7 conv is 4 fp16 matmuls of contraction 128 per 512-col chunk;
    outputs are split by weight columns into a "log_s" PSUM tile and a "t"
    PSUM tile, each [128, TS] holding BOTH samples of the pair (64-col
    matmuls auto-col-tile into PE column groups 0/64, which run
    concurrently),
  - ScalarE: TH = tanh(ps_ls + b_ls) [128,TS]; E = exp(TH*alpha+beta) fp16,
  - VectorE: ZM halves = E * x (fp16, 2x mode); then one fused
    scalar_tensor_tensor: zstage = (ps_t + b_t) + ZM,
  - z stored as fp16 (upcast to fp32 on host; well within tolerance) from
    [128, 2*TS] staging chunks as soon as both halves are written.

DMA plan: x supertile loads are ~0.5 MB transfers (8 KB per-partition
lines) -- x2a halves on sync (HWDGE), x2b halves + z stores on gpsimd
(SWDGE); constants are merged into 3 DMAs; supertile-0 zero halos are
memset, and the first pair's x tiles are issued right after the weights so
the first matmul starts early. PE warm-up matmuls run during the head's
DMA wait to flip the HAM clock gate to 8/8 before real work.
"""

import numpy as np

import concourse.bacc as bacc
import concourse.bass as bass
import concourse.mybir as mybir
import concourse.tile as tile
from concourse.bass_utils import run_bass_kernel_spmd

N_CORES = 8
B, C, T = 32, 64, 8192
NS = B // N_CORES          # samples per core
TS = 1024                  # time-tile width (multiple of 512)
SUP = 4                    # compute tiles per DMA supertile
O = 128                    # output channels (2C)

F32 = mybir.dt.float32
F16 = mybir.dt.float16


# ---------------------------------------------------------------- host math

def _compose(w_in, b_in, w_mid, b_mid, w_out, b_out):
    """W_eff (128, 64, 7), b_eff (128,), D (128, 4)."""
    w_in = np.asarray(w_in, np.float64)
    w_mid = np.asarray(w_mid, np.float64)
    w_out = np.asarray(w_out, np.float64)
    b_in = np.asarray(b_in, np.float64)
    b_mid = np.asarray(b_mid, np.float64)
    b_out = np.asarray(b_out, np.float64)
    H = w_in.shape[0]

    w12 = np.zeros((w_mid.shape[0], w_in.shape[1], 5))
    for i in range(3):
        for j in range(3):
            w12[:, :, i + j] += w_mid[:, :, j] @ w_in[:, :, i]
    w_eff = np.zeros((w_out.shape[0], w_in.shape[1], 7))
    for i in range(5):
        for l in range(3):
            w_eff[:, :, i + l] += w_out[:, :, l] @ w12[:, :, i]

    b_eff = w_out.sum(2) @ (w_mid.sum(2) @ b_in + b_mid) + b_out

    # boundary correction: reference chain on x=0, minus steady-state b_eff
    Tz = 12
    rs1 = np.zeros((H, Tz)) + b_in[:, None]
    rs2 = np.zeros((w_mid.shape[0], Tz))
    for t in range(Tz):
        acc = b_mid.copy()
        for j in range(3):
            tau = t - 2 + j
            if tau >= 0:
                acc = acc + w_mid[:, :, j] @ rs1[:, tau]
        rs2[:, t] = acc
    rs3 = np.zeros((w_out.shape[0], Tz))
    for t in range(Tz):
        acc = b_out.copy()
        for l in range(3):
            tau = t - 2 + l
            if tau >= 0:
                acc = acc + w_out[:, :, l] @ rs2[:, tau]
        rs3[:, t] = acc
    D = rs3[:, 0:4] - b_eff[:, None]
    return (w_eff.astype(np.float32), b_eff.astype(np.float32),
            D.astype(np.float32))


def _pack_weights(w_eff):
    """Stationary lhsT tiles as (p, m, o): m 0-2 = tap pairs (1,2),(3,4),
    (5,6) with rows 0-63 = odd tap, 64-127 = even tap; m 4-6 = swapped
    halves; m 8/9 = tap-0 weights for ls/t duplicated into all four 64x64
    quadrants (consumed by the 64x64-tiled quad matmuls)."""
    WT = np.zeros((10, 128, 128), np.float32)
    for m in range(3):
        WT[m, 0:64, :] = w_eff[:, :, 2 * m + 1].T
        WT[m, 64:128, :] = w_eff[:, :, 2 * m + 2].T
        WT[4 + m, 64:128, :] = w_eff[:, :, 2 * m + 1].T
        WT[4 + m, 0:64, :] = w_eff[:, :, 2 * m + 2].T
    q_ls = w_eff[0:64, :, 0].T
    q_t = w_eff[64:128, :, 0].T
    WT[8] = np.block([[q_ls, q_ls], [q_ls, q_ls]])
    WT[9] = np.block([[q_t, q_t], [q_t, q_t]])
    return np.ascontiguousarray(WT.transpose(1, 0, 2))


# ------------------------------------------------------------- device build

def build_nc(ns=NS, t_len=T, ts=TS, sup=SUP):
    assert ts % 512 == 0 and t_len % (ts * sup) == 0
    # supertile plan per pair: small leading supertiles so the first
    # matmul's x DMA is small and arrivals stay ahead of compute while
    # the rings ramp up; full-size ones in steady state
    first_plan = [1, 1, 2, 2, 2]
    rest_plan = [sup] * (t_len // ts // sup)
    assert sum(first_plan) == sum(rest_plan) == t_len // ts
    nc = bacc.Bacc("TRN2", target_bir_lowering=False, debug=False,
                   num_devices=N_CORES)
    xs = nc.dram_tensor("xs", (ns, C, t_len), F16, kind="ExternalInput").ap()
    # host-pretransposed to (p, m, o) so the load is a dense 320 KB DMA
    wt = nc.dram_tensor("wt", (128, 10, 128), F16, kind="ExternalInput").ap()
    # cmat rows: 0-127 = D_ls one-hot lhsT cols, 128-255 = D_t, 256-767 = bind
    cmat = nc.dram_tensor("cmat", (4, 768), F16, kind="ExternalInput").ap()
    # cvec cols: 0 = b_ls, 1 = b_t, 2 = alpha, 3 = beta (dup-packed halves)
    cvec = nc.dram_tensor("cvec", (128, 4), F32, kind="ExternalInput").ap()
    z = nc.dram_tensor("z", (ns, C, t_len), F16, kind="ExternalOutput").ap()

    W4 = sup * ts + 8          # max supertile width incl halo
    pairs = ns // 2
    Tanh = mybir.ActivationFunctionType.Tanh
    Exp = mybir.ActivationFunctionType.Exp
    ADD = mybir.AluOpType.add
    MUL = mybir.AluOpType.mult

    with tile.TileContext(nc) as tc:
        with (
            tc.tile_pool(name="consts", bufs=1) as consts,
            tc.tile_pool(name="data", bufs=2) as data,
            tc.tile_pool(name="outs", bufs=3) as outs,
            tc.tile_pool(name="psum", bufs=2, space="PSUM") as psum_pool,
        ):
            def load_xsuper(x2, s, t0, nsub, swapped, dma_lo, dma_hi):
                # lo rows get x shifted 8, hi rows x shifted 7; a swapped
                # tile exchanges which partition half holds which shift.
                lo = x2[64:128, :] if swapped else x2[0:64, :]
                hi = x2[0:64, :] if swapped else x2[64:128, :]
                w = nsub * ts + 8
                if t0 == 0:
                    nc.gpsimd.memset(lo[:, 0:8], 0.0)
                    nc.gpsimd.memset(hi[:, 0:7], 0.0)
                    dma_lo(lo[:, 8:w], xs[s, :, 0:w - 8])
                    dma_hi(hi[:, 7:w - 1], xs[s, :, 0:w - 8])
                else:
                    dma_lo(lo[:, 0:w], xs[s, :, t0 - 8:t0 + nsub * ts])
                    dma_hi(hi[:, 0:w - 1], xs[s, :, t0 - 7:t0 + nsub * ts])

            def make_x2(pr, t0, nsub):
                # one buffer per supertile: loads and halo memsets never
                # wait on buffer reuse, so they can never block the DMA
                # queues behind them. (The ACT queue must stay free of DMA
                # dispatches - they'd delay the tanh/exp chain.)
                x2a = data.tile([128, W4], F16, tag="x2a", name="x2a",
                                bufs=7)
                x2b = data.tile([128, W4], F16, tag="x2b", name="x2b",
                                bufs=7)
                load_xsuper(x2a, 2 * pr, t0, nsub, False,
                            nc.sync.dma_start, nc.sync.dma_start)
                load_xsuper(x2b, 2 * pr + 1, t0, nsub, True,
                            nc.gpsimd.dma_start, nc.gpsimd.dma_start)
                return x2a, x2b

            # supertile schedule: (pair, t0, nsub)
            supers = []
            for pr in range(pairs):
                plan = first_plan if pr == 0 else rest_plan
                t0 = 0
                for nsub in plan:
                    supers.append((pr, t0, nsub))
                    t0 += nsub * ts

            # weights first (the first matmul's only const dependency),
            # then the first supertiles on all three DMA rings; cmat/cvec
            # (needed only ~2us into the first iteration) go last
            wt_sb = consts.tile([128, 10, 128], F16)
            nc.sync.dma_start(wt_sb, wt)
            pre = [make_x2(*supers[0])]
            pre.append(make_x2(*supers[1]))
            cm_sb = consts.tile([4, 768], F16)
            nc.gpsimd.dma_start(cm_sb, cmat)
            cv_sb = consts.tile([128, 4], F32)
            nc.sync.dma_start(cv_sb, cvec)

            # PE warm-up: ~4.5us of throwaway matmuls during the head's
            # DMA wait flips the HAM clock gate to 8/8 before real work
            warm = data.tile([128, 512], F16, tag="warm", name="warm",
                             bufs=1)
            nc.vector.memset(warm, 0.0)
            warm_ps = psum_pool.tile([128, 512], F32, tag="psls",
                                     name="warm_ps")
            for _ in range(9):
                nc.tensor.matmul(warm_ps[:, 0:512], warm[:, 0:128],
                                 warm[:, 0:512], start=True, stop=True,
                                 skip_group_check=True)
            bls_sb = cv_sb[:, 0:1]
            btt_sb = cv_sb[:, 1:2]
            avec_sb = cv_sb[:, 2:3]
            bevec_sb = cv_sb[:, 3:4]
            dls_sb = cm_sb[:, 0:128]
            dtt_sb = cm_sb[:, 128:256]
            bind_sb = cm_sb[:, 256:768]

            xq = dict(enumerate(pre))
            zst = None
            for k, (pr, st0, nsub) in enumerate(supers):
                # prefetch upcoming supertiles' x while this one computes
                for kf in (k + 1, k + 2, k + 3):
                    if kf < len(supers) and kf not in xq:
                        xq[kf] = make_x2(*supers[kf])
                x2a, x2b = xq.pop(k)
                s0, s1 = 2 * pr, 2 * pr + 1

                for q in range(nsub):
                    it = st0 // ts + q     # tile index within the pair
                    o = q * ts             # column offset in the supertile
                    t0 = it * ts

                    ps_ls = psum_pool.tile([128, ts], F32, tag="psls",
                                           name="ps_ls")
                    ps_t = psum_pool.tile([128, ts], F32, tag="pst",
                                          name="ps_t")
                    corr = (it == 0)

                    # tap-0 runs as four concurrent 64x64-tiled K=64
                    # matmuls (2 row-bands = h-chunks x 2 col-bands =
                    # samples). The ls quad OPENS its psum (start=True,
                    # iteration head) and the t quad CLOSES its psum
                    # (stop=True, iteration tail): ps_ls still completes
                    # as early as possible for the tanh chain, while
                    # ps_t's first write happens as LATE as possible so
                    # the 2-deep psum rotation has maximal slack. Only 2
                    # tile-mode switches per iteration this way.
                    def quad(ps, mq, start, stop_h0, stop_h1):
                        w6 = wt_sb[:, mq, :]
                        nc.tensor.matmul(
                            ps[0:64, 0:512], w6[0:64, 0:64],
                            x2a[0:64, o + 1:o + 513],
                            start=start, stop=stop_h0)
                        nc.tensor.matmul(
                            ps[64:128, 0:512], w6[64:128, 64:128],
                            x2b[64:128, o + 1:o + 513],
                            start=start, stop=stop_h0)
                        nc.tensor.matmul(
                            ps[0:64, 512:1024], w6[64:128, 0:64],
                            x2a[64:128, o + 512:o + 1024],
                            start=start, stop=stop_h1)
                        nc.tensor.matmul(
                            ps[64:128, 512:1024], w6[0:64, 64:128],
                            x2b[0:64, o + 512:o + 1024],
                            start=start, stop=stop_h1)

                    def pairs(ps, o0, start, stop_last):
                        for m in range(3):
                            for h in range(ts // 512):
                                ho = h * 512
                                c0 = o + ho + 2 * m + 2
                                stop = (stop_last and m == 2
                                        and not (corr and h == 0))
                                nc.tensor.matmul(
                                    ps[0:64, ho:ho + 512],
                                    wt_sb[:, m, o0:o0 + 64],
                                    x2a[:, c0:c0 + 512],
                                    start=(start and m == 0), stop=stop)
                                nc.tensor.matmul(
                                    ps[64:128, ho:ho + 512],
                                    wt_sb[:, 4 + m, o0:o0 + 64],
                                    x2b[:, c0:c0 + 512],
                                    start=(start and m == 0), stop=stop)

                    def corr_mm(ps, o0):
                        nc.tensor.matmul(
                            ps[:, 0:512],
                            dls_sb if o0 == 0 else dtt_sb,
                            bind_sb, start=False, stop=True,
                            skip_group_check=True)

                    quad(ps_ls, 8, start=True, stop_h0=False,
                         stop_h1=False)
                    pairs(ps_ls, 0, start=False, stop_last=True)
                    if corr:
                        corr_mm(ps_ls, 0)
                    pairs(ps_t, 64, start=True, stop_last=False)
                    quad(ps_t, 9, start=False, stop_h0=not corr,
                         stop_h1=True)
                    if corr:
                        corr_mm(ps_t, 64)

                    th = outs.tile([128, ts], F16, tag="th", name="th")
                    nc.scalar.activation(th, ps_ls, Tanh, bias=bls_sb)
                    e = outs.tile([128, ts], F16, tag="e", name="e")
                    nc.scalar.activation(e, th, Exp,
                                         bias=bevec_sb, scale=avec_sb)
                    zm = outs.tile([128, ts], F16, tag="zm", name="zm")
                    nc.vector.tensor_tensor(
                        zm[0:64, :], e[0:64, :],
                        x2a[0:64, 8 + o:8 + o + ts], MUL)
                    nc.vector.tensor_tensor(
                        zm[64:128, :], e[64:128, :],
                        x2b[64:128, 8 + o:8 + o + ts], MUL)
                    # fused: zstage = (ps_t + b_t) + zm  (one DVE op)
                    if it % 2 == 0:
                        zst = outs.tile([128, 2 * ts], F16, tag="zst",
                                        name="zst")
                    zo = (it % 2) * ts
                    nc.vector.scalar_tensor_tensor(
                        zst[:, zo:zo + ts], ps_t, btt_sb, zm, ADD, ADD)
                    if it >= t_len // ts - 2:
                        # final two tiles: store each 1024-col chunk the
                        # moment it is ready, halving the tail drain
                        nc.sync.dma_start(z[s0, :, t0:t0 + ts],
                                          zst[0:64, zo:zo + ts])
                        nc.gpsimd.dma_start(z[s1, :, t0:t0 + ts],
                                            zst[64:128, zo:zo + ts])
                    elif it % 2 == 1:
                        # split stores across the two DMA queues; loads for
                        # upcoming supertiles are issued ahead of these in
                        # program order so the zstage-ready wait here never
                        # delays a load dispatch
                        tst = t0 - ts
                        nc.sync.dma_start(z[s0, :, tst:tst + 2 * ts],
                                          zst[0:64, :])
                        nc.gpsimd.dma_start(z[s1, :, tst:tst + 2 * ts],
                                            zst[64:128, :])

    nc.compile()
    return nc


def make_in_maps(x, w_in, b_in, w_mid, b_mid, w_out, b_out, alpha, beta,
                 n_cores=N_CORES):
    w_eff, b_eff, D = _compose(w_in, b_in, w_mid, b_mid, w_out, b_out)
    WT = _pack_weights(w_eff)
    CMAT = np.zeros((4, 768), np.float32)
    CMAT[:, 0:64] = D[0:64, :].T
    CMAT[:, 64:128] = D[0:64, :].T
    CMAT[:, 128:192] = D[64:128, :].T
    CMAT[:, 192:256] = D[64:128, :].T
    for i in range(4):
        CMAT[i, 256 + i] = 1.0
    al = np.asarray(alpha, np.float32).reshape(64)
    be = np.asarray(beta, np.float32).reshape(64)
    CVEC = np.stack([
        np.concatenate([b_eff[0:64], b_eff[0:64]]),
        np.concatenate([b_eff[64:128], b_eff[64:128]]),
        np.concatenate([al, al]),
        np.concatenate([be, be]),
    ], axis=1).astype(np.float32)
    x16 = np.ascontiguousarray(np.asarray(x, np.float32)).astype(np.float16)
    ns = x16.shape[0] // n_cores
    maps = []
    for i in range(n_cores):
        m = dict(xs=np.ascontiguousarray(x16[i * ns:(i + 1) * ns]),
                 wt=WT.astype(np.float16),
                 cmat=CMAT.astype(np.float16),
                 cvec=np.ascontiguousarray(CVEC))
        maps.append(m)
    return maps


_NC_CACHE = {}


def _get_nc():
    if "nc" not in _NC_CACHE:
        _NC_CACHE["nc"] = build_nc()
    return _NC_CACHE["nc"]


def kernel(x, w_in, b_in, w_mid, b_mid, w_out, b_out, alpha, beta,
           _trace=False, _trace_kwargs=None):
    nc = _get_nc()
    in_maps = make_in_maps(x, w_in, b_in, w_mid, b_mid, w_out, b_out,
                           alpha, beta)
    res = run_bass_kernel_spmd(nc, in_maps, core_ids=list(range(N_CORES)),
                               trace=_trace, **(_trace_kwargs or {}))
    out = np.concatenate([r["z"] for r in res.results], axis=0)
    kernel.last_results = res
    return out.astype(np.float32)


# revision 33
# speedup vs baseline: 1.0861x; 1.0061x over previous
"""Fused ARFlow kernel for Trainium2 (8 NeuronCores, data-parallel over batch).

Reference computes three causal K=3 convs (64->256->256->128 ch) with NO
nonlinearity between them, then z = exp(alpha*tanh(ls)+beta)*x + tt.
The convs are linear, so they compose on the host into a single causal K=7
conv (64->128 ch) with an effective bias, exact for t>=4; an x-independent
(weights-only) correction D fixes outputs t<4 where the reference's
zero-padding of *biased* intermediates differs from the composition.

Device kernel per core (4 samples, processed as 2 sample-PAIRS so every
post-matmul op runs at the full 128 partitions):
  - x is loaded per sample as a [128, SW+8] fp16 SUPERTILE (4 compute
    tiles worth) with tap-pair packing (partitions 0-63 = x shifted 8,
    64-127 = x shifted 7; the second sample of a pair uses the SWAPPED
    layout so its data sits in partitions 64-127 wherever the pair-packed
    elementwise ops need it),
  - the K=7 conv per 512-col chunk: tap 0 runs as four CONCURRENT
    64x64-tiled K=64 matmuls (2 row-bands = chunk halves x 2 col-bands =
    samples), taps 1-6 as 3 pair-packed K=128 matmuls whose 64-col
    outputs auto-col-tile into PE column groups 0/64 (samples a/b
    concurrent). Outputs split into a "log_s" PSUM tile and a "t" PSUM
    tile, each [128, TS] holding BOTH samples. The ls quad OPENS the
    iteration (start=True) and the t quad CLOSES it (stop=True) so the
    2-deep PSUM rotation has maximal slack; only 2 PE tile-mode switches
    per iteration,
  - ScalarE: TH = tanh(ps_ls + b_ls) [128,TS]; E = exp(TH*alpha+beta) fp16,
  - VectorE: ZM halves = E * x (fp16, 2x mode); then one fused
    scalar_tensor_tensor: zstage = (ps_t + b_t) + ZM,
  - z stored as fp16 (upcast to fp32 on host; well within tolerance) from
    [128, 2*TS] staging chunks as soon as both halves are written (the
    final two tiles store per-1024 chunk to shorten the tail drain).

DMA plan: x supertile loads ramp [1,1,2,2,2]x1024 cols for pair 0 (small
first arrivals gate the first matmul; ring throughput ramps up), 4x1024
afterwards; x2a on the sync HWDGE ring, x2b on the gpsimd SWDGE ring, z
stores split across both, and the ACT ring carries NO DMA dispatches
(their waits would stall the tanh/exp chain - engine queues are strictly
in-order). Every supertile gets its own buffer (bufs=7) so loads/memsets
never wait on buffer reuse and can never block later dispatches.
Supertile-0 zero halos are memset; PE warm-up matmuls during the head's
DMA wait flip the HAM clock gate to 8/8 before real work.
"""

import numpy as np

import concourse.bacc as bacc
import concourse.bass as bass
import concourse.mybir as mybir
import concourse.tile as tile
from concourse.bass_utils import run_bass_kernel_spmd

N_CORES = 8
B, C, T = 32, 64, 8192
NS = B // N_CORES          # samples per core
TS = 1024                  # time-tile width (multiple of 512)
SUP = 4                    # compute tiles per DMA supertile
O = 128                    # output channels (2C)

F32 = mybir.dt.float32
F16 = mybir.dt.float16


# ---------------------------------------------------------------- host math

def _compose(w_in, b_in, w_mid, b_mid, w_out, b_out):
    """W_eff (128, 64, 7), b_eff (128,), D (128, 4)."""
    w_in = np.asarray(w_in, np.float64)
    w_mid = np.asarray(w_mid, np.float64)
    w_out = np.asarray(w_out, np.float64)
    b_in = np.asarray(b_in, np.float64)
    b_mid = np.asarray(b_mid, np.float64)
    b_out = np.asarray(b_out, np.float64)
    H = w_in.shape[0]

    w12 = np.zeros((w_mid.shape[0], w_in.shape[1], 5))
    for i in range(3):
        for j in range(3):
            w12[:, :, i + j] += w_mid[:, :, j] @ w_in[:, :, i]
    w_eff = np.zeros((w_out.shape[0], w_in.shape[1], 7))
    for i in range(5):
        for l in range(3):
            w_eff[:, :, i + l] += w_out[:, :, l] @ w12[:, :, i]

    b_eff = w_out.sum(2) @ (w_mid.sum(2) @ b_in + b_mid) + b_out

    # boundary correction: reference chain on x=0, minus steady-state b_eff
    Tz = 12
    rs1 = np.zeros((H, Tz)) + b_in[:, None]
    rs2 = np.zeros((w_mid.shape[0], Tz))
    for t in range(Tz):
        acc = b_mid.copy()
        for j in range(3):
            tau = t - 2 + j
            if tau >= 0:
                acc = acc + w_mid[:, :, j] @ rs1[:, tau]
        rs2[:, t] = acc
    rs3 = np.zeros((w_out.shape[0], Tz))
    for t in range(Tz):
        acc = b_out.copy()
        for l in range(3):
            tau = t - 2 + l
            if tau >= 0:
                acc = acc + w_out[:, :, l] @ rs2[:, tau]
        rs3[:, t] = acc
    D = rs3[:, 0:4] - b_eff[:, None]
    return (w_eff.astype(np.float32), b_eff.astype(np.float32),
            D.astype(np.float32))


def _pack_weights(w_eff):
    """Stationary lhsT tiles as (p, m, o): m 0-2 = tap pairs (1,2),(3,4),
    (5,6) with rows 0-63 = odd tap, 64-127 = even tap; m 4-6 = swapped
    halves; m 8/9 = tap-0 weights for ls/t duplicated into all four 64x64
    quadrants (consumed by the 64x64-tiled quad matmuls)."""
    WT = np.zeros((10, 128, 128), np.float32)
    for m in range(3):
        WT[m, 0:64, :] = w_eff[:, :, 2 * m + 1].T
        WT[m, 64:128, :] = w_eff[:, :, 2 * m + 2].T
        WT[4 + m, 64:128, :] = w_eff[:, :, 2 * m + 1].T
        WT[4 + m, 0:64, :] = w_eff[:, :, 2 * m + 2].T
    q_ls = w_eff[0:64, :, 0].T
    q_t = w_eff[64:128, :, 0].T
    WT[8] = np.block([[q_ls, q_ls], [q_ls, q_ls]])
    WT[9] = np.block([[q_t, q_t], [q_t, q_t]])
    return np.ascontiguousarray(WT.transpose(1, 0, 2))


# ------------------------------------------------------------- device build

def build_nc(ns=NS, t_len=T, ts=TS, sup=SUP):
    assert ts % 512 == 0 and t_len % (ts * sup) == 0
    # supertile plan per pair: small leading supertiles so the first
    # matmul's x DMA is small and arrivals stay ahead of compute while
    # the rings ramp up; full-size ones in steady state
    first_plan = [1, 1, 2, 2, 2]
    rest_plan = [sup] * (t_len // ts // sup)
    assert sum(first_plan) == sum(rest_plan) == t_len // ts
    nc = bacc.Bacc("TRN2", target_bir_lowering=False, debug=False,
                   num_devices=N_CORES)
    xs = nc.dram_tensor("xs", (ns, C, t_len), F16, kind="ExternalInput").ap()
    # host-pretransposed to (p, m, o) so the load is a dense 320 KB DMA
    wt = nc.dram_tensor("wt", (128, 10, 128), F16, kind="ExternalInput").ap()
    # cmat rows: 0-127 = D_ls one-hot lhsT cols, 128-255 = D_t, 256-767 = bind
    cmat = nc.dram_tensor("cmat", (4, 768), F16, kind="ExternalInput").ap()
    # cvec cols: 0 = b_ls, 1 = b_t, 2 = alpha, 3 = beta (dup-packed halves)
    cvec = nc.dram_tensor("cvec", (128, 4), F32, kind="ExternalInput").ap()
    z = nc.dram_tensor("z", (ns, C, t_len), F16, kind="ExternalOutput").ap()

    W4 = sup * ts + 8          # max supertile width incl halo
    pairs = ns // 2
    Tanh = mybir.ActivationFunctionType.Tanh
    Exp = mybir.ActivationFunctionType.Exp
    ADD = mybir.AluOpType.add
    MUL = mybir.AluOpType.mult

    with tile.TileContext(nc) as tc:
        with (
            tc.tile_pool(name="consts", bufs=1) as consts,
            tc.tile_pool(name="data", bufs=2) as data,
            tc.tile_pool(name="outs", bufs=3) as outs,
            tc.tile_pool(name="psum", bufs=2, space="PSUM") as psum_pool,
        ):
            def load_xsuper(x2, s, t0, nsub, swapped, dma_lo, dma_hi):
                # lo rows get x shifted 8, hi rows x shifted 7; a swapped
                # tile exchanges which partition half holds which shift.
                lo = x2[64:128, :] if swapped else x2[0:64, :]
                hi = x2[0:64, :] if swapped else x2[64:128, :]
                w = nsub * ts + 8
                if t0 == 0:
                    nc.gpsimd.memset(lo[:, 0:8], 0.0)
                    nc.gpsimd.memset(hi[:, 0:7], 0.0)
                    dma_lo(lo[:, 8:w], xs[s, :, 0:w - 8])
                    dma_hi(hi[:, 7:w - 1], xs[s, :, 0:w - 8])
                else:
                    dma_lo(lo[:, 0:w], xs[s, :, t0 - 8:t0 + nsub * ts])
                    dma_hi(hi[:, 0:w - 1], xs[s, :, t0 - 7:t0 + nsub * ts])

            def make_x2(pr, t0, nsub):
                # one buffer per supertile: loads and halo memsets never
                # wait on buffer reuse, so they can never block the DMA
                # queues behind them. (The ACT queue must stay free of DMA
                # dispatches - they'd delay the tanh/exp chain.)
                x2a = data.tile([128, W4], F16, tag="x2a", name="x2a",
                                bufs=7)
                x2b = data.tile([128, W4], F16, tag="x2b", name="x2b",
                                bufs=7)
                load_xsuper(x2a, 2 * pr, t0, nsub, False,
                            nc.sync.dma_start, nc.sync.dma_start)
                load_xsuper(x2b, 2 * pr + 1, t0, nsub, True,
                            nc.gpsimd.dma_start, nc.gpsimd.dma_start)
                return x2a, x2b

            # supertile schedule: (pair, t0, nsub)
            supers = []
            for pr in range(pairs):
                plan = first_plan if pr == 0 else rest_plan
                t0 = 0
                for nsub in plan:
                    supers.append((pr, t0, nsub))
                    t0 += nsub * ts

            # weights first (the first matmul's only const dependency),
            # then the first supertiles on all three DMA rings; cmat/cvec
            # (needed only ~2us into the first iteration) go last
            wt_sb = consts.tile([128, 10, 128], F16)
            nc.sync.dma_start(wt_sb, wt)
            pre = [make_x2(*supers[0])]
            pre.append(make_x2(*supers[1]))
            cm_sb = consts.tile([4, 768], F16)
            nc.gpsimd.dma_start(cm_sb, cmat)
            cv_sb = consts.tile([128, 4], F32)
            nc.sync.dma_start(cv_sb, cvec)

            # PE warm-up: ~4.5us of throwaway matmuls during the head's
            # DMA wait flips the HAM clock gate to 8/8 before real work
            warm = data.tile([128, 512], F16, tag="warm", name="warm",
                             bufs=1)
            nc.vector.memset(warm, 0.0)
            warm_ps = psum_pool.tile([128, 512], F32, tag="psls",
                                     name="warm_ps")
            for _ in range(9):
                nc.tensor.matmul(warm_ps[:, 0:512], warm[:, 0:128],
                                 warm[:, 0:512], start=True, stop=True,
                                 skip_group_check=True)
            bls_sb = cv_sb[:, 0:1]
            btt_sb = cv_sb[:, 1:2]
            avec_sb = cv_sb[:, 2:3]
            bevec_sb = cv_sb[:, 3:4]
            dls_sb = cm_sb[:, 0:128]
            dtt_sb = cm_sb[:, 128:256]
            bind_sb = cm_sb[:, 256:768]

            xq = dict(enumerate(pre))
            zst = None
            for k, (pr, st0, nsub) in enumerate(supers):
                # prefetch upcoming supertiles' x while this one computes
                for kf in (k + 1, k + 2, k + 3):
                    if kf < len(supers) and kf not in xq:
                        xq[kf] = make_x2(*supers[kf])
                x2a, x2b = xq.pop(k)
                s0, s1 = 2 * pr, 2 * pr + 1

                for q in range(nsub):
                    it = st0 // ts + q     # tile index within the pair
                    o = q * ts             # column offset in the supertile
                    t0 = it * ts

                    ps_ls = psum_pool.tile([128, ts], F32, tag="psls",
                                           name="ps_ls")
                    ps_t = psum_pool.tile([128, ts], F32, tag="pst",
                                          name="ps_t")
                    corr = (it == 0)

                    # tap-0 runs as four concurrent 64x64-tiled K=64
                    # matmuls (2 row-bands = h-chunks x 2 col-bands =
                    # samples). The ls quad OPENS its psum (start=True,
                    # iteration head) and the t quad CLOSES its psum
                    # (stop=True, iteration tail): ps_ls still completes
                    # as early as possible for the tanh chain, while
                    # ps_t's first write happens as LATE as possible so
                    # the 2-deep psum rotation has maximal slack. Only 2
                    # tile-mode switches per iteration this way.
                    def quad(ps, mq, start, stop_h0, stop_h1):
                        w6 = wt_sb[:, mq, :]
                        nc.tensor.matmul(
                            ps[0:64, 0:512], w6[0:64, 0:64],
                            x2a[0:64, o + 1:o + 513],
                            start=start, stop=stop_h0)
                        nc.tensor.matmul(
                            ps[64:128, 0:512], w6[64:128, 64:128],
                            x2b[64:128, o + 1:o + 513],
                            start=start, stop=stop_h0)
                        nc.tensor.matmul(
                            ps[0:64, 512:1024], w6[64:128, 0:64],
                            x2a[64:128, o + 512:o + 1024],
                            start=start, stop=stop_h1)
                        nc.tensor.matmul(
                            ps[64:128, 512:1024], w6[0:64, 64:128],
                            x2b[0:64, o + 512:o + 1024],
                            start=start, stop=stop_h1)

                    def pairs(ps, o0, start, stop_last):
                        for m in range(3):
                            for h in range(ts // 512):
                                ho = h * 512
                                c0 = o + ho + 2 * m + 2
                                stop = (stop_last and m == 2
                                        and not (corr and h == 0))
                                nc.tensor.matmul(
                                    ps[0:64, ho:ho + 512],
                                    wt_sb[:, m, o0:o0 + 64],
                                    x2a[:, c0:c0 + 512],
                                    start=(start and m == 0), stop=stop)
                                nc.tensor.matmul(
                                    ps[64:128, ho:ho + 512],
                                    wt_sb[:, 4 + m, o0:o0 + 64],
                                    x2b[:, c0:c0 + 512],
                                    start=(start and m == 0), stop=stop)

                    def corr_mm(ps, o0):
                        nc.tensor.matmul(
                            ps[:, 0:512],
                            dls_sb if o0 == 0 else dtt_sb,
                            bind_sb, start=False, stop=True,
                            skip_group_check=True)

                    quad(ps_ls, 8, start=True, stop_h0=False,
                         stop_h1=False)
                    pairs(ps_ls, 0, start=False, stop_last=True)
                    if corr:
                        corr_mm(ps_ls, 0)
                    pairs(ps_t, 64, start=True, stop_last=False)
                    quad(ps_t, 9, start=False, stop_h0=not corr,
                         stop_h1=True)
                    if corr:
                        corr_mm(ps_t, 64)

                    th = outs.tile([128, ts], F16, tag="th", name="th")
                    nc.scalar.activation(th, ps_ls, Tanh, bias=bls_sb)
                    e = outs.tile([128, ts], F16, tag="e", name="e")
                    nc.scalar.activation(e, th, Exp,
                                         bias=bevec_sb, scale=avec_sb)
                    zm = outs.tile([128, ts], F16, tag="zm", name="zm")
                    nc.vector.tensor_tensor(
                        zm[0:64, :], e[0:64, :],
                        x2a[0:64, 8 + o:8 + o + ts], MUL)
                    nc.vector.tensor_tensor(
                        zm[64:128, :], e[64:128, :],
                        x2b[64:128, 8 + o:8 + o + ts], MUL)
                    # fused: zstage = (ps_t + b_t) + zm  (one DVE op)
                    if it % 2 == 0:
                        zst = outs.tile([128, 2 * ts], F16, tag="zst",
                                        name="zst")
                    zo = (it % 2) * ts
                    nc.vector.scalar_tensor_tensor(
                        zst[:, zo:zo + ts], ps_t, btt_sb, zm, ADD, ADD)
                    if it >= t_len // ts - 2:
                        # final two tiles: store each 1024-col chunk the
                        # moment it is ready, halving the tail drain
                        nc.sync.dma_start(z[s0, :, t0:t0 + ts],
                                          zst[0:64, zo:zo + ts])
                        nc.gpsimd.dma_start(z[s1, :, t0:t0 + ts],
                                            zst[64:128, zo:zo + ts])
                    elif it % 2 == 1:
                        # split stores across the two DMA queues; loads for
                        # upcoming supertiles are issued ahead of these in
                        # program order so the zstage-ready wait here never
                        # delays a load dispatch
                        tst = t0 - ts
                        nc.sync.dma_start(z[s0, :, tst:tst + 2 * ts],
                                          zst[0:64, :])
                        nc.gpsimd.dma_start(z[s1, :, tst:tst + 2 * ts],
                                            zst[64:128, :])

    nc.compile()
    return nc


def make_in_maps(x, w_in, b_in, w_mid, b_mid, w_out, b_out, alpha, beta,
                 n_cores=N_CORES):
    w_eff, b_eff, D = _compose(w_in, b_in, w_mid, b_mid, w_out, b_out)
    WT = _pack_weights(w_eff)
    CMAT = np.zeros((4, 768), np.float32)
    CMAT[:, 0:64] = D[0:64, :].T
    CMAT[:, 64:128] = D[0:64, :].T
    CMAT[:, 128:192] = D[64:128, :].T
    CMAT[:, 192:256] = D[64:128, :].T
    for i in range(4):
        CMAT[i, 256 + i] = 1.0
    al = np.asarray(alpha, np.float32).reshape(64)
    be = np.asarray(beta, np.float32).reshape(64)
    CVEC = np.stack([
        np.concatenate([b_eff[0:64], b_eff[0:64]]),
        np.concatenate([b_eff[64:128], b_eff[64:128]]),
        np.concatenate([al, al]),
        np.concatenate([be, be]),
    ], axis=1).astype(np.float32)
    x16 = np.ascontiguousarray(np.asarray(x, np.float32)).astype(np.float16)
    ns = x16.shape[0] // n_cores
    maps = []
    for i in range(n_cores):
        m = dict(xs=np.ascontiguousarray(x16[i * ns:(i + 1) * ns]),
                 wt=WT.astype(np.float16),
                 cmat=CMAT.astype(np.float16),
                 cvec=np.ascontiguousarray(CVEC))
        maps.append(m)
    return maps


_NC_CACHE = {}


def _get_nc():
    if "nc" not in _NC_CACHE:
        _NC_CACHE["nc"] = build_nc()
    return _NC_CACHE["nc"]


def kernel(x, w_in, b_in, w_mid, b_mid, w_out, b_out, alpha, beta,
           _trace=False, _trace_kwargs=None):
    nc = _get_nc()
    in_maps = make_in_maps(x, w_in, b_in, w_mid, b_mid, w_out, b_out,
                           alpha, beta)
    res = run_bass_kernel_spmd(nc, in_maps, core_ids=list(range(N_CORES)),
                               trace=_trace, **(_trace_kwargs or {}))
    out = np.concatenate([r["z"] for r in res.results], axis=0)
    kernel.last_results = res
    return out.astype(np.float32)
